# revision 10
# baseline (speedup 1.0000x reference)
"""Distributed Trainium2 Bass kernel: masked (upper-triangular) attention.

reference (L=4096, D=1024, fp32):
    Q = x @ Wq + bq ; K = z @ Wk + bk ; V = z @ Wv + bv
    S = Q @ K.T ; S[row > col] = -inf
    out = softmax(S / sqrt(D)) @ V

Strategy (8 NeuronCores, SPMD, ZERO collectives):
  Sequence-parallel on query rows, with every projection re-associated into
  host-side folds so each core runs only two big matmul sweeps over local
  data:
      G   = x @ Wqk + bqk        Wqk = Wq @ Wk.T / sqrt(D)   (host fp32)
      S'  = G @ z.T              (= S/sqrt(D) up to a per-query constant
                                  from bk that cancels in softmax)
      out = (exp(S')*mask @ V) / rowsum(exp(S')*mask)
            with V = z @ Wv + bv (host fp32) -- the bv term is exact because
            the unnormalized row sum divides out.
  - full z/V are inputs, so feeding them (bf16, pre-tiled) to every core
    costs no collective and no device-side transpose.
  - S^T tiles (keys on partitions): softmax output P^T chunks are the
    stationary operand of the PV matmuls, which therefore produce the output
    directly with query rows on partitions. Row sums via a ones-stationary
    matmul; the reciprocal is redistributed across partitions with a tiny
    DRAM round-trip that hides under the second PV sweep.
  - Mask applied multiplicatively after exp (scores O(1): no overflow),
    built from iota + per-core row0 input: one graph for all cores.
  - One shared PSUM pool (tag-recycled across phases, no barriers); key and
    value tiles streamed just-in-time as 512KB paired DMAs over all 3 rings.
"""

import math

import numpy as np
import ml_dtypes

import concourse.mybir as mybir
import concourse.tile as tile
from concourse import bacc
from concourse.bass_utils import run_bass_kernel_spmd

F32 = mybir.dt.float32
BF16 = mybir.dt.bfloat16
AF = mybir.ActivationFunctionType
OP = mybir.AluOpType
P = 128
NCORES = 8

L = 4096
D = 1024

BF = ml_dtypes.bfloat16


def build_graph(Ldim=L, Ddim=D):
    nc = bacc.Bacc("TRN2", target_bir_lowering=False, debug=False, num_devices=NCORES)
    ROWS = Ldim // NCORES        # query rows per core (512)
    MB = ROWS // P               # 128-row query chunks per core (4)
    KB = Ldim // P               # 128-key blocks over full z (32)
    PK = KB // 2                 # paired key blocks (16)
    IO = Ddim // P               # 128-chunks of the d dimension (8)
    DH = Ddim // 2               # value-column half width (512)
    NPRE = min(6, PK)            # V1 pairs prefetched during sweep 1

    xT_ext = nc.declare_dram_parameter("xT", [P, IO, ROWS], BF16, isOutput=False)
    wqk_ext = nc.declare_dram_parameter("wqk", [IO, P, Ddim], BF16, isOutput=False)
    zT_ext = nc.declare_dram_parameter("zTp", [PK, P, 2 * Ddim], BF16, isOutput=False)
    v0_ext = nc.declare_dram_parameter("v0p", [PK, P, 2 * DH], BF16, isOutput=False)
    v1_ext = nc.declare_dram_parameter("v1p", [PK, P, 2 * DH], BF16, isOutput=False)
    bqk_ext = nc.declare_dram_parameter("bqk", [Ddim], F32, isOutput=False)
    row0_ext = nc.declare_dram_parameter("row0", [1], F32, isOutput=False)
    out_ext = nc.declare_dram_parameter("out", [ROWS, Ddim], F32, isOutput=True)

    ones_d = nc.inline_tensor(np.ones((P, P), np.float32), name="ones_c")
    eye_d = nc.inline_tensor(np.eye(P, dtype=np.float32), name="eye_c")
    # nkb[p, kb] = -128*kb ; mask keeps where (m - p) + (row0 - 128*kb) <= 0
    nkb_d = nc.inline_tensor(
        np.broadcast_to((-float(P) * np.arange(KB, dtype=np.float32))[None, :],
                        (P, KB)).copy(), name="nkb_c")

    with tile.TileContext(nc) as tc:
        with tc.tile_pool(name="const", bufs=1) as constp, \
             tc.tile_pool(name="persist", bufs=1) as persist, \
             tc.tile_pool(name="wrot", bufs=3) as wrot, \
             tc.tile_pool(name="ktp", bufs=4) as ktp, \
             tc.tile_pool(name="vtp", bufs=3) as vtp, \
             tc.tile_pool(name="vtp2", bufs=7) as vtp2, \
             tc.tile_pool(name="osp", bufs=4) as osp, \
             tc.tile_pool(name="psp", bufs=1, space="PSUM") as psp, \
             tc.tile_pool(name="dram", bufs=1, space="DRAM") as dram:
            # PE warmup against an sbuf tile zeroed on the (otherwise idle)
            # gpsimd queue, so the HAM clock-gate opens while inputs land
            wmup = constp.tile([P, 512], BF16)
            nc.gpsimd.memset(wmup[:], 0.0)
            wpsum = psp.tile([P, 512], F32, tag="b", name="wpsum", bufs=1)
            for i in range(32):
                nc.tensor.matmul(wpsum[:], wmup[:, 0:128], wmup[:],
                                 start=True, stop=True)

            # x^T lands first, split across all three rings
            xTs = persist.tile([P, IO, ROWS], BF16)
            nc.sync.dma_start(out=xTs[:, 0:3, :], in_=xT_ext[:, 0:3, :])
            nc.scalar.dma_start(out=xTs[:, 3:6, :], in_=xT_ext[:, 3:6, :])
            nc.gpsimd.dma_start(out=xTs[:, 6:IO, :], in_=xT_ext[:, 6:IO, :])

            # small consts (scalar ring)
            ones_f = constp.tile([P, P], F32)
            nc.scalar.dma_start(out=ones_f[:], in_=ones_d.ap())
            ones128 = constp.tile([P, P], BF16)
            nc.vector.tensor_copy(ones128[:], ones_f[:])
            ident = constp.tile([P, P], F32)
            nc.gpsimd.dma_start(out=ident[:], in_=eye_d.ap())
            bqks = constp.tile([P, IO], F32)
            nc.scalar.dma_start(out=bqks[:], in_=bqk_ext[:].rearrange("(dc p) -> p dc", p=P))
            row0b = constp.tile([P, 1], F32)
            nc.scalar.dma_start(out=row0b[:], in_=row0_ext[:].partition_broadcast(P))
            nkb = constp.tile([P, KB], F32)
            nc.scalar.dma_start(out=nkb[:], in_=nkb_d.ap())
            r0kb = constp.tile([P, KB], F32)
            nc.vector.tensor_scalar(r0kb[:], nkb[:], row0b[:], None, OP.add)

            GT = persist.tile([P, IO, ROWS], BF16)
            es = persist.tile([P, KB, ROWS], BF16)
            recT = persist.tile([P, MB], F32)
            mmk = persist.tile([P, KB, ROWS], BF16)

            # ------------- Phase A: G^T = Wqk^T-chunks @ x^T + bqk -----------
            for dc in range(IO):
                wqa = wrot.tile([P, Ddim], BF16, tag="wq", name=f"wqa_{dc}")
                eng = nc.scalar if dc % 2 == 0 else nc.sync
                eng.dma_start(out=wqa[:], in_=wqk_ext[dc])
                gp = psp.tile([P, 512], F32, tag="a", name=f"gp_{dc}", bufs=2)
                for io in range(IO):
                    nc.tensor.matmul(gp[:, 0:ROWS], wqa[:, io * P:(io + 1) * P],
                                     xTs[:, io, :],
                                     start=(io == 0), stop=(io == IO - 1))
                nc.vector.tensor_scalar(GT[:, dc, :], gp[:, 0:ROWS],
                                        bqks[:, dc:dc + 1], None, OP.add)

            # masks, emitted after the projection vector-work so they fill the
            # DVE pipe during early sweep 1 without delaying G^T
            with tc.tile_pool(name="iop", bufs=1) as iop:
                iota1 = iop.tile([P, ROWS], F32)
                nc.gpsimd.iota(iota1[:], pattern=[[1, ROWS]], base=0,
                               channel_multiplier=-1,
                               allow_small_or_imprecise_dtypes=True)
                for kb in range(KB):
                    nc.vector.tensor_scalar(mmk[:, kb, :], iota1[:],
                                            r0kb[:, kb:kb + 1], 0.0,
                                            OP.add, OP.is_le)

            # ------- Phase B: S^T sweep + exp/mask + l + PV (out half 0) -----
            lps = psp.tile([P, 512], F32, tag="b", name="lps", bufs=1)
            ovA = [psp.tile([P, 512], F32, tag=f"o{mb}", name=f"ovA_{mb}", bufs=1)
                   for mb in range(MB)]
            kts = [None] * PK
            vts = [None] * PK
            vt2s = [None] * PK

            def emit_s(kb):
                pk, j = kb // 2, kb % 2
                if j == 0:
                    kt = ktp.tile([P, 2 * Ddim], BF16, tag="kt", name=f"kt_{pk}")
                    # first pairs ride gpsimd so wqk keeps sync/scalar early
                    eng = (nc.gpsimd if pk < 2
                           else (nc.sync if pk % 2 == 0 else nc.scalar))
                    eng.dma_start(out=kt[:], in_=zT_ext[pk])
                    kts[pk] = kt
                    vt = vtp.tile([P, 2 * DH], BF16, tag="vt", name=f"vt_{pk}")
                    nc.gpsimd.dma_start(out=vt[:], in_=v0_ext[pk])
                    vts[pk] = vt
                kt = kts[pk]
                sp = psp.tile([P, 512], F32, tag="a", name=f"sp_{kb}", bufs=2)
                for io in range(IO):
                    nc.tensor.matmul(
                        sp[:, 0:ROWS],
                        kt[:, j * Ddim + io * P:j * Ddim + (io + 1) * P],
                        GT[:, io, :],
                        start=(io == 0), stop=(io == IO - 1))
                nc.scalar.activation(es[:, kb, :], sp[:, 0:ROWS], AF.Exp)
                nc.vector.tensor_tensor(es[:, kb, :], es[:, kb, :],
                                        mmk[:, kb, :], OP.mult)

            def emit_lpv(kb):
                pk, j = kb // 2, kb % 2
                nc.tensor.matmul(lps[:, 0:ROWS], ones128[:], es[:, kb, :],
                                 start=(kb == 0), stop=(kb == KB - 1))
                vt = vts[pk]
                for mb in range(MB):
                    nc.tensor.matmul(ovA[mb][:],
                                     es[:, kb, mb * P:(mb + 1) * P],
                                     vt[:, j * DH:(j + 1) * DH],
                                     start=(kb == 0), stop=(kb == KB - 1))

            emit_s(0)
            emit_s(1)
            for kb in range(KB):
                emit_lpv(kb)
                if kb + 2 < KB:
                    emit_s(kb + 2)
                if kb % 2 == 0 and kb // 2 >= PK - NPRE:  # prefetch V1 pairs
                    ppk = kb // 2 - (PK - NPRE)
                    vt2 = vtp2.tile([P, 2 * DH], BF16, tag="vt2", name=f"vt2_{ppk}")
                    nc.gpsimd.dma_start(out=vt2[:], in_=v1_ext[ppk])
                    vt2s[ppk] = vt2

            # row-sums -> SBUF (all 128 lanes) -> PE transpose -> 1/l per
            # query partition; no DRAM round-trip, frees the lps bank fast
            lsb = constp.tile([P, ROWS], F32, tag="lsb", name="lsb")
            nc.vector.tensor_copy(lsb[:], lps[:, 0:ROWS])
            ltp = psp.tile([P, 512], F32, tag="b", name="ltp", bufs=1)
            for mb in range(MB):
                nc.tensor.transpose(ltp[:, mb * P:(mb + 1) * P],
                                    lsb[:, mb * P:(mb + 1) * P], ident[:])
            for mb in range(MB):
                nc.vector.reciprocal(recT[:, mb:mb + 1], ltp[:, mb * P:mb * P + 1])

            oview = out_ext[:].rearrange("(mb p) v -> p mb v", p=P)

            def emit_out(mb, h, op):
                osb = osp.tile([P, DH], F32, tag="os", name=f"os_{mb}_{h}")
                nc.vector.tensor_scalar(osb[:], op[:],
                                        recT[:, mb:mb + 1], None, OP.mult)
                nc.scalar.dma_start(out=oview[:, mb, h * DH:(h + 1) * DH], in_=osb[:])

            # ------------- Phase C: PV (out half 1) --------------------------
            # accumulators recycle sweep-1 banks; mb order puts the fresh bank
            # first so the reciprocal's read of lps never stalls the PE
            ovB = [None] * MB
            ovB[MB - 1] = psp.tile([P, 512], F32, tag="c", name="ovB_last", bufs=1)
            ovB[0] = psp.tile([P, 512], F32, tag="a", name="ovB_0", bufs=2)
            if MB > 2:
                ovB[1] = psp.tile([P, 512], F32, tag="a", name="ovB_1", bufs=2)
            if MB > 3:
                ovB[2] = psp.tile([P, 512], F32, tag="b", name="ovB_2", bufs=1)
            mb_order = [MB - 1] + list(range(MB - 1))
            for mb in range(MB):
                emit_out(mb, 0, ovA[mb])
            for kb in range(KB):
                pk, j = kb // 2, kb % 2
                if j == 0:
                    vt = vt2s[pk]
                    if vt is None:
                        vt = vtp2.tile([P, 2 * DH], BF16, tag="vt2",
                                       name=f"vt2_{pk}")
                        eng = nc.sync if pk % 2 == 0 else nc.scalar
                        eng.dma_start(out=vt[:], in_=v1_ext[pk])
                        vt2s[pk] = vt
                vt = vt2s[pk]
                for mb in mb_order:
                    nc.tensor.matmul(ovB[mb][:],
                                     es[:, kb, mb * P:(mb + 1) * P],
                                     vt[:, j * DH:(j + 1) * DH],
                                     start=(kb == 0), stop=(kb == KB - 1))
            for mb in range(MB):
                emit_out(mb, 1, ovB[mb])
    nc.compile()
    return nc


_GRAPH_CACHE = {}


def _get_graph(Ldim=L, Ddim=D):
    key = (Ldim, Ddim)
    if key not in _GRAPH_CACHE:
        _GRAPH_CACHE[key] = build_graph(Ldim, Ddim)
    return _GRAPH_CACHE[key]


def kernel(x, z, Wq, bq, Wk, bk, Wv, bv):
    x = np.ascontiguousarray(np.asarray(x, dtype=np.float32))
    z = np.ascontiguousarray(np.asarray(z, dtype=np.float32))
    Ldim, Ddim = x.shape
    nc = _get_graph(Ldim, Ddim)
    ROWS = Ldim // NCORES
    KB = Ldim // P
    PK = KB // 2
    IO = Ddim // P
    DH = Ddim // 2
    scale = 1.0 / math.sqrt(Ddim)

    Wq = np.asarray(Wq, np.float32)
    Wk = np.asarray(Wk, np.float32)
    Wv = np.asarray(Wv, np.float32)
    bq = np.asarray(bq, np.float32)
    bv = np.asarray(bv, np.float32)
    # host-side folds (fp32): Wqk = Wq Wk^T/sqrt(D); V = z Wv + bv
    Wqk = (Wq @ Wk.T) * scale
    bqk = ((bq @ Wk.T) * scale).astype(np.float32)
    V = (z @ Wv + bv).astype(np.float32)

    zT = np.ascontiguousarray(z.T).astype(BF)                      # [D, L]
    zTt = zT.reshape(IO, P, KB, P).transpose(2, 1, 0, 3).reshape(KB, P, Ddim)
    zTp = np.ascontiguousarray(
        zTt.reshape(PK, 2, P, Ddim).transpose(0, 2, 1, 3).reshape(PK, P, 2 * Ddim))
    vr = V.reshape(KB, P, Ddim).astype(BF)                         # [kb, key, v]
    v0p = np.ascontiguousarray(
        vr[:, :, :DH].reshape(PK, 2, P, DH).transpose(0, 2, 1, 3)
        .reshape(PK, P, 2 * DH))
    v1p = np.ascontiguousarray(
        vr[:, :, DH:].reshape(PK, 2, P, DH).transpose(0, 2, 1, 3)
        .reshape(PK, P, 2 * DH))
    # per-output-block layout: wqk[dc] holds one 128-column output block of
    # Wqk across all contraction chunks
    wqk_a = np.ascontiguousarray(
        Wqk.reshape(IO, P, IO, P).transpose(2, 1, 0, 3).reshape(IO, P, Ddim)
    ).astype(BF)

    common = {
        "wqk": wqk_a, "zTp": zTp, "v0p": v0p, "v1p": v1p,
        "bqk": np.ascontiguousarray(bqk),
    }
    in_maps = []
    for c in range(NCORES):
        m = dict(common)
        xc = x[ROWS * c:ROWS * (c + 1)]
        m["xT"] = np.ascontiguousarray(
            xc.T.reshape(IO, P, ROWS).transpose(1, 0, 2)).astype(BF)
        m["row0"] = np.array([ROWS * c], dtype=np.float32)
        in_maps.append(m)
    try:
        res = run_bass_kernel_spmd(nc, in_maps, core_ids=list(range(NCORES)))
    except Exception:
        # transient NRT device hiccups have been observed; one retry
        res = run_bass_kernel_spmd(nc, in_maps, core_ids=list(range(NCORES)))
    out = np.empty((Ldim, Ddim), dtype=np.float32)
    for c in range(NCORES):
        out[ROWS * c:ROWS * (c + 1)] = res.results[c]["out"]
    return out


# revision 11
# speedup vs baseline: 1.0435x; 1.0435x over previous
"""Distributed Trainium2 Bass kernel: masked (upper-triangular) attention.

reference (L=4096, D=1024, fp32):
    Q = x @ Wq + bq ; K = z @ Wk + bk ; V = z @ Wv + bv
    S = Q @ K.T ; S[row > col] = -inf
    out = softmax(S / sqrt(D)) @ V

Strategy (8 NeuronCores, SPMD, ZERO collectives):
  Sequence-parallel on query rows, with every projection re-associated into
  host-side folds so each core runs only two big matmul sweeps over local
  data:
      G   = x @ Wqk + bqk        Wqk = Wq @ Wk.T / sqrt(D)   (host fp32)
      S'  = G @ z.T              (= S/sqrt(D) up to a per-query constant
                                  from bk that cancels in softmax)
      out = (exp(S')*mask @ V) / rowsum(exp(S')*mask)
            with V = z @ Wv + bv (host fp32) -- the bv term is exact because
            the unnormalized row sum divides out.
  - full z/V are inputs, so feeding them (bf16, pre-tiled) to every core
    costs no collective and no device-side transpose.
  - S^T tiles (keys on partitions): softmax output P^T chunks are the
    stationary operand of the PV matmuls, which therefore produce the output
    directly with query rows on partitions. Row sums via a ones-stationary
    matmul; the reciprocal is redistributed across partitions with a tiny
    DRAM round-trip that hides under the second PV sweep.
  - Mask applied multiplicatively after exp (scores O(1): no overflow),
    built from iota + per-core row0 input: one graph for all cores.
  - One shared PSUM pool (tag-recycled across phases, no barriers); key and
    value tiles streamed just-in-time as 512KB paired DMAs over all 3 rings.
"""

import math

import numpy as np
import ml_dtypes

import concourse.mybir as mybir
import concourse.tile as tile
from concourse import bacc
from concourse.bass_utils import run_bass_kernel_spmd

F32 = mybir.dt.float32
BF16 = mybir.dt.bfloat16
AF = mybir.ActivationFunctionType
OP = mybir.AluOpType
P = 128
NCORES = 8

L = 4096
D = 1024

BF = ml_dtypes.bfloat16


def build_graph(Ldim=L, Ddim=D):
    nc = bacc.Bacc("TRN2", target_bir_lowering=False, debug=False, num_devices=NCORES)
    ROWS = Ldim // NCORES        # query rows per core (512)
    MB = ROWS // P               # 128-row query chunks per core (4)
    KB = Ldim // P               # 128-key blocks over full z (32)
    PK = KB // 2                 # paired key blocks (16)
    IO = Ddim // P               # 128-chunks of the d dimension (8)
    DH = Ddim // 2               # value-column half width (512)
    NPRE = min(6, PK)            # V1 pairs prefetched during sweep 1

    xT_ext = nc.declare_dram_parameter("xT", [P, IO, ROWS], BF16, isOutput=False)
    wqk_ext = nc.declare_dram_parameter("wqk", [IO, P, Ddim], BF16, isOutput=False)
    zT_ext = nc.declare_dram_parameter("zTp", [PK, P, 2 * Ddim], BF16, isOutput=False)
    v0_ext = nc.declare_dram_parameter("v0p", [PK, P, 2 * DH], BF16, isOutput=False)
    v1_ext = nc.declare_dram_parameter("v1p", [PK, P, 2 * DH], BF16, isOutput=False)
    cst_ext = nc.declare_dram_parameter("cst", [P, KB + IO], F32, isOutput=False)
    ones_ext = nc.declare_dram_parameter("onesb", [P, P], BF16, isOutput=False)
    eye_ext = nc.declare_dram_parameter("eye", [P, P], F32, isOutput=False)
    out_ext = nc.declare_dram_parameter("out", [ROWS, Ddim], F32, isOutput=True)

    with tile.TileContext(nc) as tc:
        with tc.tile_pool(name="const", bufs=1) as constp, \
             tc.tile_pool(name="persist", bufs=1) as persist, \
             tc.tile_pool(name="wrot", bufs=3) as wrot, \
             tc.tile_pool(name="ktp", bufs=4) as ktp, \
             tc.tile_pool(name="vtp", bufs=3) as vtp, \
             tc.tile_pool(name="vtp2", bufs=7) as vtp2, \
             tc.tile_pool(name="osp", bufs=4) as osp, \
             tc.tile_pool(name="psp", bufs=1, space="PSUM") as psp, \
             tc.tile_pool(name="dram", bufs=1, space="DRAM") as dram:
            # PE warmup against an sbuf tile zeroed on the (otherwise idle)
            # gpsimd queue, so the HAM clock-gate opens while inputs land
            wmup = constp.tile([P, 512], BF16)
            nc.gpsimd.memset(wmup[:], 0.0)
            wpsum = psp.tile([P, 512], F32, tag="b", name="wpsum", bufs=1)
            for i in range(32):
                nc.tensor.matmul(wpsum[:], wmup[:, 0:128], wmup[:],
                                 start=True, stop=True)

            # x^T lands first, split across all three rings
            xTs = persist.tile([P, IO, ROWS], BF16)
            nc.sync.dma_start(out=xTs[:, 0:IO // 2, :], in_=xT_ext[:, 0:IO // 2, :])
            nc.gpsimd.dma_start(out=xTs[:, IO // 2:IO, :], in_=xT_ext[:, IO // 2:IO, :])

            # host-prepared consts: cst = [r0kb | bqks], contiguous per
            # partition (no tiny-packet rearrange/broadcast DMAs)
            cst = constp.tile([P, KB + IO], F32)
            nc.sync.dma_start(out=cst[:], in_=cst_ext[:])
            r0kb = cst[:, 0:KB]
            bqks = cst[:, KB:KB + IO]
            ones128 = constp.tile([P, P], BF16)
            nc.sync.dma_start(out=ones128[:], in_=ones_ext[:])
            ident = constp.tile([P, P], F32)

            GT = persist.tile([P, IO, ROWS], BF16)
            es = persist.tile([P, KB, ROWS], BF16)
            recT = persist.tile([P, MB], F32)
            mmk = persist.tile([P, KB, ROWS], BF16)

            # ------------- Phase A: G^T = Wqk^T-chunks @ x^T + bqk -----------
            for dc in range(IO):
                wqa = wrot.tile([P, Ddim], BF16, tag="wq", name=f"wqa_{dc}")
                eng = nc.scalar if dc % 2 == 0 else nc.sync
                eng.dma_start(out=wqa[:], in_=wqk_ext[dc])
                gp = psp.tile([P, 512], F32, tag="a", name=f"gp_{dc}", bufs=2)
                for io in range(IO):
                    nc.tensor.matmul(gp[:, 0:ROWS], wqa[:, io * P:(io + 1) * P],
                                     xTs[:, io, :],
                                     start=(io == 0), stop=(io == IO - 1))
                nc.vector.tensor_scalar(GT[:, dc, :], gp[:, 0:ROWS],
                                        cst[:, KB + dc:KB + dc + 1], None, OP.add)

            # masks, emitted after the projection vector-work so they fill the
            # DVE pipe during early sweep 1 without delaying G^T
            with tc.tile_pool(name="iop", bufs=1) as iop:
                iota1 = iop.tile([P, ROWS], F32)
                nc.gpsimd.iota(iota1[:], pattern=[[1, ROWS]], base=0,
                               channel_multiplier=-1,
                               allow_small_or_imprecise_dtypes=True)
                for kb in range(KB):
                    nc.vector.tensor_scalar(mmk[:, kb, :], iota1[:],
                                            cst[:, kb:kb + 1], 0.0,
                                            OP.add, OP.is_le)

            # ------- Phase B: S^T sweep + exp/mask + l + PV (out half 0) -----
            lps = psp.tile([P, 512], F32, tag="b", name="lps", bufs=1)
            ovA = [psp.tile([P, 512], F32, tag=f"o{mb}", name=f"ovA_{mb}", bufs=1)
                   for mb in range(MB)]
            kts = [None] * PK
            vts = [None] * PK
            vt2s = [None] * PK

            def emit_s(kb):
                pk, j = kb // 2, kb % 2
                if j == 0:
                    kt = ktp.tile([P, 2 * Ddim], BF16, tag="kt", name=f"kt_{pk}")
                    # first pairs ride gpsimd so wqk keeps sync/scalar early
                    eng = (nc.gpsimd if pk < 2
                           else (nc.sync if pk % 2 == 0 else nc.scalar))
                    eng.dma_start(out=kt[:], in_=zT_ext[pk])
                    kts[pk] = kt
                    vt = vtp.tile([P, 2 * DH], BF16, tag="vt", name=f"vt_{pk}")
                    nc.gpsimd.dma_start(out=vt[:], in_=v0_ext[pk])
                    vts[pk] = vt
                kt = kts[pk]
                sp = psp.tile([P, 512], F32, tag="a", name=f"sp_{kb}", bufs=2)
                for io in range(IO):
                    nc.tensor.matmul(
                        sp[:, 0:ROWS],
                        kt[:, j * Ddim + io * P:j * Ddim + (io + 1) * P],
                        GT[:, io, :],
                        start=(io == 0), stop=(io == IO - 1))
                nc.scalar.activation(es[:, kb, :], sp[:, 0:ROWS], AF.Exp)
                nc.vector.tensor_tensor(es[:, kb, :], es[:, kb, :],
                                        mmk[:, kb, :], OP.mult)

            def emit_lpv(kb):
                pk, j = kb // 2, kb % 2
                nc.tensor.matmul(lps[:, 0:ROWS], ones128[:], es[:, kb, :],
                                 start=(kb == 0), stop=(kb == KB - 1))
                vt = vts[pk]
                for mb in range(MB):
                    nc.tensor.matmul(ovA[mb][:],
                                     es[:, kb, mb * P:(mb + 1) * P],
                                     vt[:, j * DH:(j + 1) * DH],
                                     start=(kb == 0), stop=(kb == KB - 1))

            emit_s(0)
            emit_s(1)
            for kb in range(KB):
                emit_lpv(kb)
                if kb + 2 < KB:
                    emit_s(kb + 2)
                if kb == KB // 2:
                    nc.gpsimd.dma_start(out=ident[:], in_=eye_ext[:])
                if kb % 2 == 0 and kb // 2 >= PK - NPRE:  # prefetch V1 pairs
                    ppk = kb // 2 - (PK - NPRE)
                    vt2 = vtp2.tile([P, 2 * DH], BF16, tag="vt2", name=f"vt2_{ppk}")
                    nc.gpsimd.dma_start(out=vt2[:], in_=v1_ext[ppk])
                    vt2s[ppk] = vt2

            # row-sums -> SBUF (all 128 lanes) -> PE transpose -> 1/l per
            # query partition; no DRAM round-trip, frees the lps bank fast
            lsb = constp.tile([P, ROWS], F32, tag="lsb", name="lsb")
            nc.vector.tensor_copy(lsb[:], lps[:, 0:ROWS])
            ltp = psp.tile([P, 512], F32, tag="b", name="ltp", bufs=1)
            for mb in range(MB):
                nc.tensor.transpose(ltp[:, mb * P:(mb + 1) * P],
                                    lsb[:, mb * P:(mb + 1) * P], ident[:])
            for mb in range(MB):
                nc.vector.reciprocal(recT[:, mb:mb + 1], ltp[:, mb * P:mb * P + 1])

            oview = out_ext[:].rearrange("(mb p) v -> p mb v", p=P)

            def emit_out(mb, h, op):
                osb = osp.tile([P, DH], F32, tag="os", name=f"os_{mb}_{h}")
                nc.vector.tensor_scalar(osb[:], op[:],
                                        recT[:, mb:mb + 1], None, OP.mult)
                nc.scalar.dma_start(out=oview[:, mb, h * DH:(h + 1) * DH], in_=osb[:])

            # ------------- Phase C: PV (out half 1) --------------------------
            # accumulators recycle sweep-1 banks; mb order puts the fresh bank
            # first so the reciprocal's read of lps never stalls the PE
            ovB = [None] * MB
            ovB[MB - 1] = psp.tile([P, 512], F32, tag="c", name="ovB_last", bufs=1)
            ovB[0] = psp.tile([P, 512], F32, tag="a", name="ovB_0", bufs=2)
            if MB > 2:
                ovB[1] = psp.tile([P, 512], F32, tag="a", name="ovB_1", bufs=2)
            if MB > 3:
                ovB[2] = psp.tile([P, 512], F32, tag="b", name="ovB_2", bufs=1)
            mb_order = [MB - 1] + list(range(MB - 1))
            for mb in range(MB):
                emit_out(mb, 0, ovA[mb])
            for kb in range(KB):
                pk, j = kb // 2, kb % 2
                if j == 0:
                    vt = vt2s[pk]
                    if vt is None:
                        vt = vtp2.tile([P, 2 * DH], BF16, tag="vt2",
                                       name=f"vt2_{pk}")
                        eng = nc.sync if pk % 2 == 0 else nc.scalar
                        eng.dma_start(out=vt[:], in_=v1_ext[pk])
                        vt2s[pk] = vt
                vt = vt2s[pk]
                for mb in mb_order:
                    nc.tensor.matmul(ovB[mb][:],
                                     es[:, kb, mb * P:(mb + 1) * P],
                                     vt[:, j * DH:(j + 1) * DH],
                                     start=(kb == 0), stop=(kb == KB - 1))
            for mb in range(MB):
                emit_out(mb, 1, ovB[mb])
    nc.compile()
    return nc


_GRAPH_CACHE = {}


def _get_graph(Ldim=L, Ddim=D):
    key = (Ldim, Ddim)
    if key not in _GRAPH_CACHE:
        _GRAPH_CACHE[key] = build_graph(Ldim, Ddim)
    return _GRAPH_CACHE[key]


def kernel(x, z, Wq, bq, Wk, bk, Wv, bv):
    x = np.ascontiguousarray(np.asarray(x, dtype=np.float32))
    z = np.ascontiguousarray(np.asarray(z, dtype=np.float32))
    Ldim, Ddim = x.shape
    nc = _get_graph(Ldim, Ddim)
    ROWS = Ldim // NCORES
    KB = Ldim // P
    PK = KB // 2
    IO = Ddim // P
    DH = Ddim // 2
    scale = 1.0 / math.sqrt(Ddim)

    Wq = np.asarray(Wq, np.float32)
    Wk = np.asarray(Wk, np.float32)
    Wv = np.asarray(Wv, np.float32)
    bq = np.asarray(bq, np.float32)
    bv = np.asarray(bv, np.float32)
    # host-side folds (fp32): Wqk = Wq Wk^T/sqrt(D); V = z Wv + bv
    Wqk = (Wq @ Wk.T) * scale
    bqk = ((bq @ Wk.T) * scale).astype(np.float32)
    V = (z @ Wv + bv).astype(np.float32)

    zT = np.ascontiguousarray(z.T).astype(BF)                      # [D, L]
    zTt = zT.reshape(IO, P, KB, P).transpose(2, 1, 0, 3).reshape(KB, P, Ddim)
    zTp = np.ascontiguousarray(
        zTt.reshape(PK, 2, P, Ddim).transpose(0, 2, 1, 3).reshape(PK, P, 2 * Ddim))
    vr = V.reshape(KB, P, Ddim).astype(BF)                         # [kb, key, v]
    v0p = np.ascontiguousarray(
        vr[:, :, :DH].reshape(PK, 2, P, DH).transpose(0, 2, 1, 3)
        .reshape(PK, P, 2 * DH))
    v1p = np.ascontiguousarray(
        vr[:, :, DH:].reshape(PK, 2, P, DH).transpose(0, 2, 1, 3)
        .reshape(PK, P, 2 * DH))
    # per-output-block layout: wqk[dc] holds one 128-column output block of
    # Wqk across all contraction chunks
    wqk_a = np.ascontiguousarray(
        Wqk.reshape(IO, P, IO, P).transpose(2, 1, 0, 3).reshape(IO, P, Ddim)
    ).astype(BF)

    common = {
        "wqk": wqk_a, "zTp": zTp, "v0p": v0p, "v1p": v1p,
        "onesb": np.ones((P, P), ml_dtypes.bfloat16),
        "eye": np.eye(P, dtype=np.float32),
    }
    bqks_h = np.ascontiguousarray(bqk.reshape(IO, P).T)            # [P, IO]
    nkb_h = -float(P) * np.arange(KB, dtype=np.float32)[None, :]   # [1, KB]
    in_maps = []
    for c in range(NCORES):
        m = dict(common)
        xc = x[ROWS * c:ROWS * (c + 1)]
        m["xT"] = np.ascontiguousarray(
            xc.T.reshape(IO, P, ROWS).transpose(1, 0, 2)).astype(BF)
        # cst = [row0 - 128*kb (bcast over partitions) | bqk blocks]
        r0kb_h = np.broadcast_to(ROWS * c + nkb_h, (P, KB))
        m["cst"] = np.ascontiguousarray(
            np.concatenate([r0kb_h, bqks_h], axis=1).astype(np.float32))
        in_maps.append(m)
    try:
        res = run_bass_kernel_spmd(nc, in_maps, core_ids=list(range(NCORES)))
    except Exception:
        # transient NRT device hiccups have been observed; one retry
        res = run_bass_kernel_spmd(nc, in_maps, core_ids=list(range(NCORES)))
    out = np.empty((Ldim, Ddim), dtype=np.float32)
    for c in range(NCORES):
        out[ROWS * c:ROWS * (c + 1)] = res.results[c]["out"]
    return out


# revision 13
# speedup vs baseline: 1.0908x; 1.0454x over previous
"""Distributed Trainium2 Bass kernel: masked (upper-triangular) attention.

reference (L=4096, D=1024, fp32):
    Q = x @ Wq + bq ; K = z @ Wk + bk ; V = z @ Wv + bv
    S = Q @ K.T ; S[row > col] = -inf
    out = softmax(S / sqrt(D)) @ V

Strategy (8 NeuronCores, SPMD, ZERO collectives):
  Sequence-parallel on query rows, with every projection re-associated into
  host-side folds so each core runs only two big matmul sweeps over local
  data:
      G   = x @ Wqk + bqk        Wqk = Wq @ Wk.T / sqrt(D)   (host fp32)
      S'  = G @ z.T              (= S/sqrt(D) up to a per-query constant
                                  from bk that cancels in softmax)
      out = (exp(S')*mask @ V) / rowsum(exp(S')*mask)
            with V = z @ Wv + bv (host fp32) -- the bv term is exact because
            the unnormalized row sum divides out.
  - full z/V are inputs, so feeding them (bf16, pre-tiled) to every core
    costs no collective and no device-side transpose.
  - S^T tiles (keys on partitions): softmax output P^T chunks are the
    stationary operand of the PV matmuls, which therefore produce the output
    directly with query rows on partitions. Row sums via a ones-stationary
    matmul; the reciprocal is redistributed across partitions with a tiny
    DRAM round-trip that hides under the second PV sweep.
  - Mask applied multiplicatively after exp (scores O(1): no overflow),
    built from iota + per-core row0 input: one graph for all cores.
  - One shared PSUM pool (tag-recycled across phases, no barriers); key and
    value tiles streamed just-in-time as 512KB paired DMAs over all 3 rings.
"""

import math

import numpy as np
import ml_dtypes

import concourse.mybir as mybir
import concourse.tile as tile
from concourse import bacc
from concourse.bass_utils import run_bass_kernel_spmd

F32 = mybir.dt.float32
BF16 = mybir.dt.bfloat16
AF = mybir.ActivationFunctionType
OP = mybir.AluOpType
P = 128
NCORES = 8

L = 4096
D = 1024

BF = ml_dtypes.bfloat16


def build_graph(Ldim=L, Ddim=D):
    nc = bacc.Bacc("TRN2", target_bir_lowering=False, debug=False, num_devices=NCORES)
    ROWS = Ldim // NCORES        # query rows per core (512)
    MB = ROWS // P               # 128-row query chunks per core (4)
    KB = Ldim // P               # 128-key blocks over full z (32)
    PK = KB // 2                 # paired key blocks (16)
    IO = Ddim // P               # 128-chunks of the d dimension (8)
    DH = Ddim // 2               # value-column half width (512)
    NPRE = min(6, PK)            # V1 pairs prefetched during sweep 1

    xT_ext = nc.declare_dram_parameter("xT", [P, IO, ROWS], BF16, isOutput=False)
    wqk_ext = nc.declare_dram_parameter("wqk", [IO // 2, P, 2 * Ddim], BF16, isOutput=False)
    zT_ext = nc.declare_dram_parameter("zTp", [PK, P, 2 * Ddim], BF16, isOutput=False)
    v0_ext = nc.declare_dram_parameter("v0q", [KB // 4, P, 4 * DH], BF16, isOutput=False)
    v1_ext = nc.declare_dram_parameter("v1q", [KB // 4, P, 4 * DH], BF16, isOutput=False)
    cst_ext = nc.declare_dram_parameter("cst", [P, KB + IO], F32, isOutput=False)
    ones_ext = nc.declare_dram_parameter("onesb", [P, P], BF16, isOutput=False)
    eye_ext = nc.declare_dram_parameter("eye", [P, P], F32, isOutput=False)
    out_ext = nc.declare_dram_parameter("out", [ROWS, Ddim], F32, isOutput=True)

    with tile.TileContext(nc) as tc:
        with tc.tile_pool(name="const", bufs=1) as constp, \
             tc.tile_pool(name="persist", bufs=1) as persist, \
             tc.tile_pool(name="wrot", bufs=3) as wrot, \
             tc.tile_pool(name="ktp", bufs=5) as ktp, \
             tc.tile_pool(name="vtp", bufs=3) as vtp, \
             tc.tile_pool(name="vtp2", bufs=8) as vtp2, \
             tc.tile_pool(name="osp", bufs=4) as osp, \
             tc.tile_pool(name="psp", bufs=1, space="PSUM") as psp, \
             tc.tile_pool(name="dram", bufs=1, space="DRAM") as dram:
            # PE warmup against an sbuf tile zeroed on the (otherwise idle)
            # gpsimd queue, so the HAM clock-gate opens while inputs land
            wmup = constp.tile([P, 512], BF16)
            nc.gpsimd.memset(wmup[:], 0.0)
            wpsum = psp.tile([P, 512], F32, tag="b", name="wpsum", bufs=1)
            for i in range(20):
                nc.tensor.matmul(wpsum[:], wmup[:, 0:128], wmup[:],
                                 start=True, stop=True)

            # x^T lands first, split across all three rings
            xTs = persist.tile([P, IO, ROWS], BF16)
            nc.sync.dma_start(out=xTs[:, 0:IO // 2, :], in_=xT_ext[:, 0:IO // 2, :])
            nc.gpsimd.dma_start(out=xTs[:, IO // 2:IO, :], in_=xT_ext[:, IO // 2:IO, :])

            # host-prepared consts: cst = [r0kb | bqks], contiguous per
            # partition (no tiny-packet rearrange/broadcast DMAs)
            cst = constp.tile([P, KB + IO], F32)
            nc.sync.dma_start(out=cst[:], in_=cst_ext[:])
            r0kb = cst[:, 0:KB]
            bqks = cst[:, KB:KB + IO]
            ones128 = constp.tile([P, P], BF16)
            nc.sync.dma_start(out=ones128[:], in_=ones_ext[:])
            ident = constp.tile([P, P], F32)

            GT = persist.tile([P, IO, ROWS], BF16)
            es = persist.tile([P, KB, ROWS], BF16)
            recT = persist.tile([P, MB], F32)
            mmk = persist.tile([P, KB, ROWS], BF16)

            # ------------- Phase A: G^T = Wqk^T-chunks @ x^T + bqk -----------
            wqps = [None] * (IO // 2)
            for dc in range(IO):
                if dc % 2 == 0:
                    wqp = wrot.tile([P, 2 * Ddim], BF16, tag="wq",
                                    name=f"wqp_{dc // 2}")
                    eng = nc.scalar if (dc // 2) % 2 == 0 else nc.sync
                    eng.dma_start(out=wqp[:], in_=wqk_ext[dc // 2])
                    wqps[dc // 2] = wqp
                wqp = wqps[dc // 2]
                jw = (dc % 2) * Ddim
                gp = psp.tile([P, 512], F32, tag="a", name=f"gp_{dc}", bufs=2)
                for io in range(IO):
                    nc.tensor.matmul(gp[:, 0:ROWS],
                                     wqp[:, jw + io * P:jw + (io + 1) * P],
                                     xTs[:, io, :],
                                     start=(io == 0), stop=(io == IO - 1))
                nc.vector.tensor_scalar(GT[:, dc, :], gp[:, 0:ROWS],
                                        cst[:, KB + dc:KB + dc + 1], None, OP.add)

            # masks, emitted after the projection vector-work so they fill the
            # DVE pipe during early sweep 1 without delaying G^T
            with tc.tile_pool(name="iop", bufs=1) as iop:
                iota1 = iop.tile([P, ROWS], F32)
                nc.gpsimd.iota(iota1[:], pattern=[[1, ROWS]], base=0,
                               channel_multiplier=-1,
                               allow_small_or_imprecise_dtypes=True)
                for kb in range(KB):
                    nc.vector.tensor_scalar(mmk[:, kb, :], iota1[:],
                                            cst[:, kb:kb + 1], 0.0,
                                            OP.add, OP.is_le)

            # ------- Phase B: S^T sweep + exp/mask + l + PV (out half 0) -----
            lps = psp.tile([P, 512], F32, tag="b", name="lps", bufs=1)
            ovA = [psp.tile([P, 512], F32, tag=f"o{mb}", name=f"ovA_{mb}", bufs=1)
                   for mb in range(MB)]
            kts = [None] * PK
            vts = [None] * (KB // 4)
            vt2s = [None] * (KB // 4)

            def emit_s(kb):
                pk, j = kb // 2, kb % 2
                if j == 0:
                    kt = ktp.tile([P, 2 * Ddim], BF16, tag="kt", name=f"kt_{pk}")
                    # first pairs ride gpsimd so wqk keeps sync/scalar early
                    eng = (nc.gpsimd if pk < 2
                           else (nc.sync if pk % 2 == 0 else nc.scalar))
                    eng.dma_start(out=kt[:], in_=zT_ext[pk])
                    kts[pk] = kt
                if kb % 4 == 0:
                    vt = vtp.tile([P, 4 * DH], BF16, tag="vt", name=f"vt_{kb // 4}")
                    nc.gpsimd.dma_start(out=vt[:], in_=v0_ext[kb // 4])
                    vts[kb // 4] = vt
                kt = kts[pk]
                sp = psp.tile([P, 512], F32, tag="a", name=f"sp_{kb}", bufs=2)
                for io in range(IO):
                    nc.tensor.matmul(
                        sp[:, 0:ROWS],
                        kt[:, j * Ddim + io * P:j * Ddim + (io + 1) * P],
                        GT[:, io, :],
                        start=(io == 0), stop=(io == IO - 1))
                nc.scalar.activation(es[:, kb, :], sp[:, 0:ROWS], AF.Exp)
                nc.vector.tensor_tensor(es[:, kb, :], es[:, kb, :],
                                        mmk[:, kb, :], OP.mult)

            def emit_lpv(kb):
                j4 = kb % 4
                nc.tensor.matmul(lps[:, 0:ROWS], ones128[:], es[:, kb, :],
                                 start=(kb == 0), stop=(kb == KB - 1))
                vt = vts[kb // 4]
                for mb in range(MB):
                    nc.tensor.matmul(ovA[mb][:],
                                     es[:, kb, mb * P:(mb + 1) * P],
                                     vt[:, j4 * DH:(j4 + 1) * DH],
                                     start=(kb == 0), stop=(kb == KB - 1))

            emit_s(0)
            emit_s(1)
            emit_s(2)
            for kb in range(KB):
                emit_lpv(kb)
                if kb + 3 < KB:
                    emit_s(kb + 3)
                if kb == KB // 2:
                    nc.gpsimd.dma_start(out=ident[:], in_=eye_ext[:])
                if kb % 4 == 0:                  # prefetch ALL V1 quads
                    qq = kb // 4
                    vt2 = vtp2.tile([P, 4 * DH], BF16, tag="vt2", name=f"vt2_{qq}")
                    nc.gpsimd.dma_start(out=vt2[:], in_=v1_ext[qq])
                    vt2s[qq] = vt2

            # row-sums -> SBUF (all 128 lanes) -> PE transpose -> 1/l per
            # query partition; no DRAM round-trip, frees the lps bank fast
            lsb = constp.tile([P, ROWS], F32, tag="lsb", name="lsb")
            nc.vector.tensor_copy(lsb[:], lps[:, 0:ROWS])
            ltp = psp.tile([P, 512], F32, tag="b", name="ltp", bufs=1)
            for mb in range(MB):
                nc.tensor.transpose(ltp[:, mb * P:(mb + 1) * P],
                                    lsb[:, mb * P:(mb + 1) * P], ident[:])
            for mb in range(MB):
                nc.vector.reciprocal(recT[:, mb:mb + 1], ltp[:, mb * P:mb * P + 1])

            oview = out_ext[:].rearrange("(mb p) v -> p mb v", p=P)

            def emit_out(mb, h, op):
                osb = osp.tile([P, DH], F32, tag="os", name=f"os_{mb}_{h}")
                nc.vector.tensor_scalar(osb[:], op[:],
                                        recT[:, mb:mb + 1], None, OP.mult)
                nc.scalar.dma_start(out=oview[:, mb, h * DH:(h + 1) * DH], in_=osb[:])

            # ------------- Phase C: PV (out half 1) --------------------------
            # accumulators recycle sweep-1 banks; mb order puts the fresh bank
            # first so the reciprocal's read of lps never stalls the PE
            ovB = [None] * MB
            ovB[MB - 1] = psp.tile([P, 512], F32, tag="c", name="ovB_last", bufs=1)
            ovB[0] = psp.tile([P, 512], F32, tag="a", name="ovB_0", bufs=2)
            if MB > 2:
                ovB[1] = psp.tile([P, 512], F32, tag="a", name="ovB_1", bufs=2)
            if MB > 3:
                ovB[2] = psp.tile([P, 512], F32, tag="b", name="ovB_2", bufs=1)
            mb_order = [MB - 1] + list(range(MB - 1))
            for mb in range(MB):
                emit_out(mb, 0, ovA[mb])
            for mb in mb_order:
                for kb in range(KB):
                    j4 = kb % 4
                    vt = vt2s[kb // 4]
                    nc.tensor.matmul(ovB[mb][:],
                                     es[:, kb, mb * P:(mb + 1) * P],
                                     vt[:, j4 * DH:(j4 + 1) * DH],
                                     start=(kb == 0), stop=(kb == KB - 1))
                emit_out(mb, 1, ovB[mb])
    nc.compile()
    return nc


_GRAPH_CACHE = {}


def _get_graph(Ldim=L, Ddim=D):
    key = (Ldim, Ddim)
    if key not in _GRAPH_CACHE:
        _GRAPH_CACHE[key] = build_graph(Ldim, Ddim)
    return _GRAPH_CACHE[key]


def kernel(x, z, Wq, bq, Wk, bk, Wv, bv):
    x = np.ascontiguousarray(np.asarray(x, dtype=np.float32))
    z = np.ascontiguousarray(np.asarray(z, dtype=np.float32))
    Ldim, Ddim = x.shape
    nc = _get_graph(Ldim, Ddim)
    ROWS = Ldim // NCORES
    KB = Ldim // P
    PK = KB // 2
    IO = Ddim // P
    DH = Ddim // 2
    scale = 1.0 / math.sqrt(Ddim)

    Wq = np.asarray(Wq, np.float32)
    Wk = np.asarray(Wk, np.float32)
    Wv = np.asarray(Wv, np.float32)
    bq = np.asarray(bq, np.float32)
    bv = np.asarray(bv, np.float32)
    # host-side folds (fp32): Wqk = Wq Wk^T/sqrt(D); V = z Wv + bv
    Wqk = (Wq @ Wk.T) * scale
    bqk = ((bq @ Wk.T) * scale).astype(np.float32)
    V = (z @ Wv + bv).astype(np.float32)

    zT = np.ascontiguousarray(z.T).astype(BF)                      # [D, L]
    zTt = zT.reshape(IO, P, KB, P).transpose(2, 1, 0, 3).reshape(KB, P, Ddim)
    zTp = np.ascontiguousarray(
        zTt.reshape(PK, 2, P, Ddim).transpose(0, 2, 1, 3).reshape(PK, P, 2 * Ddim))
    vr = V.reshape(KB, P, Ddim).astype(BF)                         # [kb, key, v]
    v0q = np.ascontiguousarray(
        vr[:, :, :DH].reshape(KB // 4, 4, P, DH).transpose(0, 2, 1, 3)
        .reshape(KB // 4, P, 4 * DH))
    v1q = np.ascontiguousarray(
        vr[:, :, DH:].reshape(KB // 4, 4, P, DH).transpose(0, 2, 1, 3)
        .reshape(KB // 4, P, 4 * DH))
    # per-output-block layout: wqk[dc] holds one 128-column output block of
    # Wqk across all contraction chunks
    wqk_a = np.ascontiguousarray(
        Wqk.reshape(IO, P, IO, P).transpose(2, 1, 0, 3).reshape(IO, P, Ddim)
    ).astype(BF)
    wqk_p = np.ascontiguousarray(
        wqk_a.reshape(IO // 2, 2, P, Ddim).transpose(0, 2, 1, 3)
        .reshape(IO // 2, P, 2 * Ddim))

    common = {
        "wqk": wqk_p, "zTp": zTp, "v0q": v0q, "v1q": v1q,
        "onesb": np.ones((P, P), ml_dtypes.bfloat16),
        "eye": np.eye(P, dtype=np.float32),
    }
    bqks_h = np.ascontiguousarray(bqk.reshape(IO, P).T)            # [P, IO]
    nkb_h = -float(P) * np.arange(KB, dtype=np.float32)[None, :]   # [1, KB]
    in_maps = []
    for c in range(NCORES):
        m = dict(common)
        xc = x[ROWS * c:ROWS * (c + 1)]
        m["xT"] = np.ascontiguousarray(
            xc.T.reshape(IO, P, ROWS).transpose(1, 0, 2)).astype(BF)
        # cst = [row0 - 128*kb (bcast over partitions) | bqk blocks]
        r0kb_h = np.broadcast_to(ROWS * c + nkb_h, (P, KB))
        m["cst"] = np.ascontiguousarray(
            np.concatenate([r0kb_h, bqks_h], axis=1).astype(np.float32))
        in_maps.append(m)
    try:
        res = run_bass_kernel_spmd(nc, in_maps, core_ids=list(range(NCORES)))
    except Exception:
        # transient NRT device hiccups have been observed; one retry
        res = run_bass_kernel_spmd(nc, in_maps, core_ids=list(range(NCORES)))
    out = np.empty((Ldim, Ddim), dtype=np.float32)
    for c in range(NCORES):
        out[ROWS * c:ROWS * (c + 1)] = res.results[c]["out"]
    return out


# revision 14
# speedup vs baseline: 1.4413x; 1.3213x over previous
"""Distributed Trainium2 Bass kernel: masked (upper-triangular) attention.

reference (L=4096, D=1024, fp32):
    Q = x @ Wq + bq ; K = z @ Wk + bk ; V = z @ Wv + bv
    S = Q @ K.T ; S[row > col] = -inf
    out = softmax(S / sqrt(D)) @ V

Strategy (8 NeuronCores, SPMD, ZERO collectives):
  INTERLEAVED sequence-parallel queries: core c owns rows {c, c+8, c+16, ...}.
  Every core's mask is then structurally identical -- its query chunk mb
  (128 rows, global stride 8) only attends key blocks kb >= 8*mb -- so one
  static graph skips the fully-masked 37.5% of the score/PV work on every
  core, and the diagonal boundary is handled by a per-core mask input.
  All projections are re-associated into host-side fp32 folds:
      G   = x_c @ Wqk + bqk      Wqk = Wq @ Wk.T / sqrt(D)
      S'  = G @ z.T              (bk's per-query constant cancels in softmax)
      out = (exp(S')*mask @ V) / rowsum(exp(S')*mask),  V = z Wv + bv (exact:
            the unnormalized row sum divides the bv term out)
  - full z/V are inputs (bf16, pre-tiled by the host): no collectives, no
    device transposes.
  - S^T tiles (keys on partitions): masked exp(S') chunks are the stationary
    operand of the PV matmuls, which produce the output directly with query
    rows on partitions; row sums ride the sweep as a ones-stationary matmul
    and are transposed to query partitions by the PE.
  - Sweep 1 walks key blocks DESCENDING (wide tiles first) so the paired/
    quadded just-in-time DMA streams stay ahead of the narrow tail; PV for
    the second value half runs as per-query-chunk passes whose normalize +
    store hide under the next pass.  One shared PSUM pool; all constants
    host-packed contiguous (no tiny-packet DMAs).
"""

import math

import numpy as np
import ml_dtypes

import concourse.mybir as mybir
import concourse.tile as tile
from concourse import bacc
from concourse.bass_utils import run_bass_kernel_spmd

F32 = mybir.dt.float32
BF16 = mybir.dt.bfloat16
AF = mybir.ActivationFunctionType
OP = mybir.AluOpType
P = 128
NCORES = 8

L = 4096
D = 1024

BF = ml_dtypes.bfloat16


def build_graph(Ldim=L, Ddim=D):
    nc = bacc.Bacc("TRN2", target_bir_lowering=False, debug=False, num_devices=NCORES)
    ROWS = Ldim // NCORES        # query rows per core (512)
    MB = ROWS // P               # 128-row query chunks per core (4)
    KB = Ldim // P               # 128-key blocks over full z (32)
    PK = KB // 2                 # paired key blocks (16)
    NQ = KB // 4                 # quadded value blocks (8)
    IO = Ddim // P               # 128-chunks of the d dimension (8)
    DH = Ddim // 2               # value-column half width (512)

    def nwid(kb):                # live query columns for key block kb
        return min(ROWS, P * (kb // NCORES + 1))

    xT_ext = nc.declare_dram_parameter("xT", [P, IO, ROWS], BF16, isOutput=False)
    wqk_ext = nc.declare_dram_parameter("wqk", [IO // 2, P, 2 * Ddim], BF16, isOutput=False)
    zT_ext = nc.declare_dram_parameter("zTp", [PK, P, 2 * Ddim], BF16, isOutput=False)
    v0_ext = nc.declare_dram_parameter("v0q", [NQ, P, 4 * DH], BF16, isOutput=False)
    v1_ext = nc.declare_dram_parameter("v1q", [NQ, P, 4 * DH], BF16, isOutput=False)
    cst_ext = nc.declare_dram_parameter("cst", [P, KB + IO], F32, isOutput=False)
    ones_ext = nc.declare_dram_parameter("onesb", [P, P], BF16, isOutput=False)
    eye_ext = nc.declare_dram_parameter("eye", [P, P], F32, isOutput=False)
    out_ext = nc.declare_dram_parameter("out", [ROWS, Ddim], F32, isOutput=True)

    with tile.TileContext(nc) as tc:
        with tc.tile_pool(name="const", bufs=1) as constp, \
             tc.tile_pool(name="persist", bufs=1) as persist, \
             tc.tile_pool(name="wrot", bufs=3) as wrot, \
             tc.tile_pool(name="ktp", bufs=5) as ktp, \
             tc.tile_pool(name="vtp", bufs=3) as vtp, \
             tc.tile_pool(name="vtp2", bufs=8) as vtp2, \
             tc.tile_pool(name="osp", bufs=4) as osp, \
             tc.tile_pool(name="psp", bufs=1, space="PSUM") as psp:
            # host-prepared consts first (tiny, contiguous); the ones matrix
            # doubles as the PE-warmup operand
            cst = constp.tile([P, KB + IO], F32)
            nc.sync.dma_start(out=cst[:], in_=cst_ext[:])
            ones128 = constp.tile([P, P], BF16)
            nc.sync.dma_start(out=ones128[:], in_=ones_ext[:])
            ident = constp.tile([P, P], F32)
            wpsum = psp.tile([P, 512], F32, tag="b", name="wpsum", bufs=1)
            for i in range(20):
                nc.tensor.matmul(wpsum[:, 0:128], ones128[:], ones128[:],
                                 start=True, stop=True)

            # x^T in two big-descriptor halves on sync+gpsimd
            xTs = persist.tile([P, IO, ROWS], BF16)
            nc.sync.dma_start(out=xTs[:, 0:IO // 2, :], in_=xT_ext[:, 0:IO // 2, :])
            nc.gpsimd.dma_start(out=xTs[:, IO // 2:IO, :], in_=xT_ext[:, IO // 2:IO, :])

            GT = persist.tile([P, IO, ROWS], BF16)
            es = persist.tile([P, KB, ROWS], BF16)
            recT = persist.tile([P, MB], F32)
            mmk = persist.tile([P, KB, ROWS], BF16)

            # ------------- Phase A: G^T = Wqk^T-chunks @ x^T + bqk -----------
            wqps = [None] * (IO // 2)
            for dc in range(IO):
                if dc % 2 == 0:
                    wqp = wrot.tile([P, 2 * Ddim], BF16, tag="wq",
                                    name=f"wqp_{dc // 2}")
                    eng = nc.scalar if (dc // 2) % 2 == 0 else nc.sync
                    eng.dma_start(out=wqp[:], in_=wqk_ext[dc // 2])
                    wqps[dc // 2] = wqp
                wqp = wqps[dc // 2]
                jw = (dc % 2) * Ddim
                gp = psp.tile([P, 512], F32, tag="a", name=f"gp_{dc}", bufs=2)
                for io in range(IO):
                    nc.tensor.matmul(gp[:, 0:ROWS],
                                     wqp[:, jw + io * P:jw + (io + 1) * P],
                                     xTs[:, io, :],
                                     start=(io == 0), stop=(io == IO - 1))
                nc.vector.tensor_scalar(GT[:, dc, :], gp[:, 0:ROWS],
                                        cst[:, KB + dc:KB + dc + 1], None, OP.add)

            # masks: keep where (8m - p) + (c - 128*kb) <= 0, width nwid(kb)
            with tc.tile_pool(name="iop", bufs=1) as iop:
                iota8 = iop.tile([P, ROWS], F32)
                nc.gpsimd.iota(iota8[:], pattern=[[NCORES, ROWS]], base=0,
                               channel_multiplier=-1,
                               allow_small_or_imprecise_dtypes=True)
                for kb in range(KB):
                    nc.vector.tensor_scalar(mmk[:, kb, 0:nwid(kb)],
                                            iota8[:, 0:nwid(kb)],
                                            cst[:, kb:kb + 1], 0.0,
                                            OP.add, OP.is_le)

            # ------- Phase B: S^T sweep (descending kb) + l + PV half 0 ------
            lps = psp.tile([P, 512], F32, tag="b", name="lps", bufs=1)
            ovA = [psp.tile([P, 512], F32, tag=f"o{mb}", name=f"ovA_{mb}", bufs=1)
                   for mb in range(MB)]
            kts = [None] * PK
            vts = [None] * NQ
            vt2s = [None] * NQ
            ktn = [0]

            def emit_s(kb):
                pk = kb // 2
                n = nwid(kb)
                if kts[pk] is None:
                    kt = ktp.tile([P, 2 * Ddim], BF16, tag="kt", name=f"kt_{pk}")
                    # first fetched pairs ride gpsimd (sync/scalar carry wqk)
                    eng = (nc.gpsimd if ktn[0] < 2
                           else (nc.sync if ktn[0] % 2 == 0 else nc.scalar))
                    ktn[0] += 1
                    eng.dma_start(out=kt[:], in_=zT_ext[pk])
                    kts[pk] = kt
                if vts[kb // 4] is None:
                    vt = vtp.tile([P, 4 * DH], BF16, tag="vt", name=f"vt_{kb // 4}")
                    nc.gpsimd.dma_start(out=vt[:], in_=v0_ext[kb // 4])
                    vts[kb // 4] = vt
                kt = kts[pk]
                jk = (kb % 2) * Ddim
                sp = psp.tile([P, 512], F32, tag="a", name=f"sp_{kb}", bufs=2)
                for io in range(IO):
                    nc.tensor.matmul(
                        sp[:, 0:n],
                        kt[:, jk + io * P:jk + (io + 1) * P],
                        GT[:, io, 0:n],
                        start=(io == 0), stop=(io == IO - 1))
                nc.scalar.activation(es[:, kb, 0:n], sp[:, 0:n], AF.Exp)
                nc.vector.tensor_tensor(es[:, kb, 0:n], es[:, kb, 0:n],
                                        mmk[:, kb, 0:n], OP.mult)

            def emit_lpv(kb):
                n = nwid(kb)
                j4 = kb % 4
                nc.tensor.matmul(lps[:, 0:n], ones128[:], es[:, kb, 0:n],
                                 start=(kb == KB - 1), stop=(kb == 0))
                vt = vts[kb // 4]
                for mb in range(n // P):
                    nc.tensor.matmul(ovA[mb][:],
                                     es[:, kb, mb * P:(mb + 1) * P],
                                     vt[:, j4 * DH:(j4 + 1) * DH],
                                     start=(kb == KB - 1),
                                     stop=(kb == NCORES * mb))

            ks = list(range(KB))[::-1]
            emit_s(ks[0])
            emit_s(ks[1])
            emit_s(ks[2])
            for i, kb in enumerate(ks):
                emit_lpv(kb)
                if i + 3 < KB:
                    emit_s(ks[i + 3])
                if i == KB // 2:
                    nc.gpsimd.dma_start(out=ident[:], in_=eye_ext[:])
                if i % 8 == 0 and i // 8 < NQ // 2:  # prefetch top V1 quads
                    qq = NQ - 1 - i // 8
                    vt2 = vtp2.tile([P, 4 * DH], BF16, tag="vt2", name=f"vt2_{qq}")
                    nc.gpsimd.dma_start(out=vt2[:], in_=v1_ext[qq])
                    vt2s[qq] = vt2

            # row-sums -> SBUF (all 128 lanes) -> PE transpose -> 1/l per
            # query partition; no DRAM round-trip, frees the lps bank fast
            lsb = constp.tile([P, ROWS], F32, tag="lsb", name="lsb")
            nc.vector.tensor_copy(lsb[:], lps[:, 0:ROWS])
            ltp = psp.tile([P, 512], F32, tag="b", name="ltp", bufs=1)
            for mb in range(MB):
                nc.tensor.transpose(ltp[:, mb * P:(mb + 1) * P],
                                    lsb[:, mb * P:(mb + 1) * P], ident[:])
            for mb in range(MB):
                nc.vector.reciprocal(recT[:, mb:mb + 1], ltp[:, mb * P:mb * P + 1])

            oview = out_ext[:].rearrange("(mb p) v -> p mb v", p=P)

            def emit_out(mb, h, op):
                osb = osp.tile([P, DH], F32, tag="os", name=f"os_{mb}_{h}")
                nc.vector.tensor_scalar(osb[:], op[:],
                                        recT[:, mb:mb + 1], None, OP.mult)
                nc.scalar.dma_start(out=oview[:, mb, h * DH:(h + 1) * DH], in_=osb[:])

            # ------------- Phase C: PV (out half 1), per-chunk passes --------
            # remaining V1 quads stream in while the prefetched top passes run
            for qq in range(NQ - NQ // 2):
                vt2 = vtp2.tile([P, 4 * DH], BF16, tag="vt2", name=f"vt2_{qq}")
                eng = nc.sync if qq % 2 == 0 else nc.scalar
                eng.dma_start(out=vt2[:], in_=v1_ext[qq])
                vt2s[qq] = vt2
            ovB = [None] * MB
            ovB[MB - 1] = psp.tile([P, 512], F32, tag="c", name="ovB_last", bufs=1)
            ovB[0] = psp.tile([P, 512], F32, tag="a", name="ovB_0", bufs=2)
            if MB > 2:
                ovB[1] = psp.tile([P, 512], F32, tag="a", name="ovB_1", bufs=2)
            if MB > 3:
                ovB[2] = psp.tile([P, 512], F32, tag="b", name="ovB_2", bufs=1)
            for mb in range(MB):
                emit_out(mb, 0, ovA[mb])
            for mb in range(MB - 1, -1, -1):
                for kb in range(NCORES * mb, KB):
                    j4 = kb % 4
                    nc.tensor.matmul(ovB[mb][:],
                                     es[:, kb, mb * P:(mb + 1) * P],
                                     vt2s[kb // 4][:, j4 * DH:(j4 + 1) * DH],
                                     start=(kb == NCORES * mb),
                                     stop=(kb == KB - 1))
                emit_out(mb, 1, ovB[mb])
    nc.compile()
    return nc


_GRAPH_CACHE = {}


def _get_graph(Ldim=L, Ddim=D):
    key = (Ldim, Ddim)
    if key not in _GRAPH_CACHE:
        _GRAPH_CACHE[key] = build_graph(Ldim, Ddim)
    return _GRAPH_CACHE[key]


def kernel(x, z, Wq, bq, Wk, bk, Wv, bv):
    x = np.ascontiguousarray(np.asarray(x, dtype=np.float32))
    z = np.ascontiguousarray(np.asarray(z, dtype=np.float32))
    Ldim, Ddim = x.shape
    nc = _get_graph(Ldim, Ddim)
    ROWS = Ldim // NCORES
    KB = Ldim // P
    PK = KB // 2
    NQ = KB // 4
    IO = Ddim // P
    DH = Ddim // 2
    scale = 1.0 / math.sqrt(Ddim)

    Wq = np.asarray(Wq, np.float32)
    Wk = np.asarray(Wk, np.float32)
    Wv = np.asarray(Wv, np.float32)
    bq = np.asarray(bq, np.float32)
    bv = np.asarray(bv, np.float32)
    # host-side folds (fp32): Wqk = Wq Wk^T/sqrt(D); V = z Wv + bv
    Wqk = (Wq @ Wk.T) * scale
    bqk = ((bq @ Wk.T) * scale).astype(np.float32)
    V = (z @ Wv + bv).astype(np.float32)

    zT = np.ascontiguousarray(z.T).astype(BF)                      # [D, L]
    zTt = zT.reshape(IO, P, KB, P).transpose(2, 1, 0, 3).reshape(KB, P, Ddim)
    zTp = np.ascontiguousarray(
        zTt.reshape(PK, 2, P, Ddim).transpose(0, 2, 1, 3).reshape(PK, P, 2 * Ddim))
    vr = V.reshape(KB, P, Ddim).astype(BF)                         # [kb, key, v]
    v0q = np.ascontiguousarray(
        vr[:, :, :DH].reshape(NQ, 4, P, DH).transpose(0, 2, 1, 3)
        .reshape(NQ, P, 4 * DH))
    v1q = np.ascontiguousarray(
        vr[:, :, DH:].reshape(NQ, 4, P, DH).transpose(0, 2, 1, 3)
        .reshape(NQ, P, 4 * DH))
    # per-output-block pairs: wqk[j] holds two 128-column output blocks of
    # Wqk across all contraction chunks
    wqk_a = Wqk.reshape(IO, P, IO, P).transpose(2, 1, 0, 3).reshape(IO, P, Ddim)
    wqk_p = np.ascontiguousarray(
        wqk_a.reshape(IO // 2, 2, P, Ddim).transpose(0, 2, 1, 3)
        .reshape(IO // 2, P, 2 * Ddim)).astype(BF)

    common = {
        "wqk": wqk_p, "zTp": zTp, "v0q": v0q, "v1q": v1q,
        "onesb": np.ones((P, P), ml_dtypes.bfloat16),
        "eye": np.eye(P, dtype=np.float32),
    }
    bqks_h = np.ascontiguousarray(bqk.reshape(IO, P).T)            # [P, IO]
    nkb_h = -float(P) * np.arange(KB, dtype=np.float32)[None, :]   # [1, KB]
    in_maps = []
    for c in range(NCORES):
        m = dict(common)
        xc = x[c::NCORES]                                          # interleaved
        m["xT"] = np.ascontiguousarray(
            xc.T.reshape(IO, P, ROWS).transpose(1, 0, 2)).astype(BF)
        # cst = [c - 128*kb (bcast over partitions) | bqk blocks]
        r0kb_h = np.broadcast_to(float(c) + nkb_h, (P, KB))
        m["cst"] = np.ascontiguousarray(
            np.concatenate([r0kb_h, bqks_h], axis=1).astype(np.float32))
        in_maps.append(m)
    try:
        res = run_bass_kernel_spmd(nc, in_maps, core_ids=list(range(NCORES)))
    except Exception:
        # transient NRT device hiccups have been observed; one retry
        res = run_bass_kernel_spmd(nc, in_maps, core_ids=list(range(NCORES)))
    out = np.empty((Ldim, Ddim), dtype=np.float32)
    for c in range(NCORES):
        out[c::NCORES] = res.results[c]["out"]
    return out
